# revision 10
# baseline (speedup 1.0000x reference)
"""AdaptiveDensityTokenizer on 8 TRN2 NeuronCores.

Strategy: the memory-bound importance MLP (reads all of `features`,
64 MB) is K-sharded across the 8 cores (4096 points/core); each core
computes softplus(relu(feat^T @ W1 + b1) @ W2 + b2) for its points.
The tiny data-dependent sequential logic (spatial bucketing, per-region
farthest-point sampling of ~256 total picks, concat/scatter) runs on
host, exactly replicating the reference semantics. The final token
projection (256 tokens x 256 feat @ Wa) is also host-side (0.03% of
the FLOPs).
"""

import os
import sys

import numpy as np

for _p in ("/opt/trn_rl_repo", "/root/.axon_site/_ro/trn_rl_repo"):
    if os.path.isdir(_p) and _p not in sys.path:
        sys.path.append(_p)

NCORES = 8
B, K, C, D = 2, 32768, 256, 256
T_TOK = 256
RPD = 3
R3 = 27
KC = K // NCORES          # points per core
CH = C // 128             # contraction chunks of 128
H = C // 2                # hidden width = 128
PT = 512                  # points per matmul tile (one PSUM bank of f32)

_NC = None                # cached compiled Bass program
LAST = None               # last BassKernelResults (for profiling)


def _build_mlp():
    """One SPMD program: per-core importance MLP over a K-shard."""
    from concourse import bacc, mybir, tile

    f32 = mybir.dt.float32
    nc = bacc.Bacc(None, target_bir_lowering=False, debug=False)

    feat = nc.declare_dram_parameter("features", [B, C, KC], f32, False)
    w1 = nc.declare_dram_parameter("W1", [C, H], f32, False)
    b1 = nc.declare_dram_parameter("b1", [H, 1], f32, False)
    w2 = nc.declare_dram_parameter("W2", [H, 1], f32, False)
    out = nc.declare_dram_parameter("z", [B, KC], f32, True)

    with tile.TileContext(nc) as tc:
        with (
            tc.tile_pool(name="wpool", bufs=1) as wpool,
            tc.tile_pool(name="fpool", bufs=1) as fpool,
            tc.tile_pool(name="hpool", bufs=4) as hpool,
            tc.tile_pool(name="opool", bufs=4) as opool,
            tc.tile_pool(name="ph", bufs=4, space="PSUM") as phpool,
            tc.tile_pool(name="pi", bufs=4, space="PSUM") as pipool,
        ):
            w1t = []
            for c in range(CH):
                t = wpool.tile([128, H], f32, tag=f"w1_{c}")
                nc.sync.dma_start(out=t[:], in_=w1[c * 128:(c + 1) * 128, :])
                w1t.append(t)
            w2t = wpool.tile([H, 1], f32, tag="w2")
            nc.sync.dma_start(out=w2t[:], in_=w2[:, :])
            b1t = wpool.tile([H, 1], f32, tag="b1")
            nc.sync.dma_start(out=b1t[:], in_=b1[:, :])

            # Stage the whole 8 MB feature shard in SBUF via wide DMAs
            # (16 KB contiguous rows split into 4 column chunks so the
            # transfers spread across DMA queues).
            ftiles = {}
            for b in range(B):
                for c in range(CH):
                    t = fpool.tile([128, KC], f32, tag=f"f_{b}_{c}")
                    for q in range(4):
                        sl = slice(q * (KC // 4), (q + 1) * (KC // 4))
                        nc.sync.dma_start(
                            out=t[:, sl],
                            in_=feat[b, c * 128:(c + 1) * 128, sl],
                        )
                    ftiles[b, c] = t

            relu = mybir.ActivationFunctionType.Relu
            for b in range(B):
                for i in range(KC // PT):
                    sl = slice(i * PT, (i + 1) * PT)
                    ph = phpool.tile([H, PT], f32)
                    for c in range(CH):
                        nc.tensor.matmul(
                            ph[:],
                            w1t[c][:],
                            ftiles[b, c][:, sl],
                            start=(c == 0),
                            stop=(c == CH - 1),
                        )
                    hs = hpool.tile([H, PT], f32)
                    nc.scalar.activation(hs[:], ph[:], relu, bias=b1t[:])
                    pi = pipool.tile([1, PT], f32)
                    nc.tensor.matmul(pi[:], w2t[:], hs[:], start=True, stop=True)
                    oi = opool.tile([1, PT], f32)
                    nc.vector.tensor_copy(oi[:], pi[:])
                    nc.sync.dma_start(out=out[b:b + 1, sl], in_=oi[:])

    nc.compile()
    return nc


def _get_nc():
    global _NC
    if _NC is None:
        _NC = _build_mlp()
    return _NC


def _ensure_profile_hook():
    """Shim antenv.axon_hooks (absent in this image) so the trace=True
    path of run_bass_kernel_spmd can capture NTFF profiles, and stub the
    S3 artifact upload. Only used when BASS_PROFILE=1."""
    import types

    try:
        import antenv.axon_hooks  # noqa: F401
    except ImportError:
        try:
            import antenv
            from trn_agent_boot.trn_boot import _ntff_profile_via_ctypes

            hook = _ntff_profile_via_ctypes("/opt/axon/libaxon_pjrt.so")
            mod = types.ModuleType("antenv.axon_hooks")
            mod.get_axon_ntff_profile_hook = lambda: hook
            mod.set_axon_ntff_profile_hook = lambda h: None
            sys.modules["antenv.axon_hooks"] = mod
            antenv.axon_hooks = mod
        except Exception:
            return False
    import concourse.bass_utils as bu

    bu.upload_artifacts = lambda d: "file://" + d
    return True


def _device_importance(features, W1, b1, W2, b2):
    from concourse.bass_utils import run_bass_kernel_spmd

    global LAST
    nc = _get_nc()
    profile = bool(int(os.environ.get("BASS_PROFILE", "0")))
    if profile:
        profile = _ensure_profile_hook()
    w1 = np.ascontiguousarray(W1, np.float32)
    b1c = np.ascontiguousarray(b1, np.float32).reshape(H, 1)
    w2 = np.ascontiguousarray(W2, np.float32).reshape(H, 1)
    in_maps = []
    for core in range(NCORES):
        sl = slice(core * KC, (core + 1) * KC)
        in_maps.append({
            "features": np.ascontiguousarray(features[:, :, sl], np.float32),
            "W1": w1, "b1": b1c, "W2": w2,
        })
    res = run_bass_kernel_spmd(
        nc, in_maps, core_ids=list(range(NCORES)), trace=profile,
    )
    LAST = res
    z = np.concatenate([res.results[c]["z"] for c in range(NCORES)], axis=1)
    z = z + np.float32(np.asarray(b2).reshape(()))
    # softplus on host with the exact jax.nn.softplus formula in f32
    # (the ACT engine's Softplus LUT is not precise enough for the
    # data-dependent n_r rounding margins).
    return np.maximum(z, 0) + np.log1p(np.exp(-np.abs(z)))


def _fps_region(pts, n_steps):
    """Farthest-point sampling over one compacted region, mirroring the
    reference: start at subset index 0, squared L2, first-max argmax."""
    n = pts.shape[0]
    picks = np.empty(n_steps, np.int64)
    mind = np.full(n, np.float32(1e10), np.float32)
    p = 0
    x, y, z = pts[:, 0], pts[:, 1], pts[:, 2]
    for s in range(n_steps):
        picks[s] = p
        dx = x - x[p]
        dy = y - y[p]
        dz = z - z[p]
        d = (dx * dx + dy * dy) + dz * dz
        np.minimum(mind, d, out=mind)
        p = int(np.argmax(mind))
    return picks


def kernel(xyz, features, W1, b1, W2, b2, Wa, ba):
    xyz = np.asarray(xyz, np.float32)
    features = np.asarray(features, np.float32)
    Wa = np.asarray(Wa, np.float32)
    ba = np.asarray(ba, np.float32)

    imp = _device_importance(features, W1, b1, W2, b2)        # (B, K)

    # ---- spatial bucketing (exact reference semantics, f32 ops) ----
    mn = xyz.min(axis=1, keepdims=True)
    mx = xyz.max(axis=1, keepdims=True)
    xn = (xyz - mn) / (mx - mn + np.float32(1e-6))
    ridx = np.clip(xn * np.float32(RPD), 0, RPD - 1).astype(np.int32)
    rid = ridx[..., 0] * RPD * RPD + ridx[..., 1] * RPD + ridx[..., 2]
    valid = np.abs(xyz).sum(-1) > 0                           # (B, K)

    onehot = (rid[..., None] == np.arange(R3)) & valid[..., None]
    counts = onehot.sum(axis=1).astype(np.int32)              # (B, R3)
    reg_imp = np.einsum(
        "bk,bkr->br",
        (imp * valid).astype(np.float32),
        onehot.astype(np.float32),
    )
    share = reg_imp / (reg_imp.sum(-1, keepdims=True) + np.float32(1e-8))
    n_r = np.round(share * np.float32(T_TOK)).astype(np.int32)
    c_r = np.where(n_r == 0, 0, np.minimum(n_r, counts))      # (B, R3)

    # ---- per-region selection: FPS picks or ascending slab ----
    out_idx = np.zeros((B, T_TOK), np.int32)
    filled = np.zeros((B, T_TOK), bool)
    for b in range(B):
        start = 0
        for r in range(R3):
            c = int(c_r[b, r])
            if c == 0:
                continue
            members = np.nonzero(onehot[b, :, r])[0]          # ascending
            if counts[b, r] <= n_r[b, r]:
                sel = members[:c]
            else:
                pts = xyz[b][members]
                sel = members[_fps_region(pts, c)]
            take = min(c, T_TOK - start)
            if take > 0:
                out_idx[b, start:start + take] = sel[:take]
                filled[b, start:start + take] = True
            start += c
            if start >= T_TOK:
                break

    # ---- gather + output heads ----
    xyz_tok = np.where(
        filled[..., None], np.take_along_axis(xyz, out_idx[..., None], axis=1), 0.0
    ).astype(np.float32)

    gath = np.stack([features[b][:, out_idx[b]].T for b in range(B)])  # (B,T,C)
    tok = gath @ Wa + ba                                               # (B,T,D)
    feat_tok = np.where(filled[..., None], tok, 0.0).transpose(0, 2, 1)
    return xyz_tok.astype(np.float32), feat_tok.astype(np.float32)


# revision 12
# speedup vs baseline: 1.7497x; 1.7497x over previous
"""AdaptiveDensityTokenizer on 8 TRN2 NeuronCores.

Strategy: the memory-bound importance MLP (reads all of `features`,
64 MB) is K-sharded across the 8 cores (4096 points/core); each core
computes softplus(relu(feat^T @ W1 + b1) @ W2 + b2) for its points.
The tiny data-dependent sequential logic (spatial bucketing, per-region
farthest-point sampling of ~256 total picks, concat/scatter) runs on
host, exactly replicating the reference semantics. The final token
projection (256 tokens x 256 feat @ Wa) is also host-side (0.03% of
the FLOPs).
"""

import os
import sys

import numpy as np

for _p in ("/opt/trn_rl_repo", "/root/.axon_site/_ro/trn_rl_repo"):
    if os.path.isdir(_p) and _p not in sys.path:
        sys.path.append(_p)

NCORES = 8
B, K, C, D = 2, 32768, 256, 256
T_TOK = 256
RPD = 3
R3 = 27
KC = K // NCORES          # points per core
CH = C // 128             # contraction chunks of 128
H = C // 2                # hidden width = 128
PT = 512                  # points per matmul tile (one PSUM bank of f32)

_NC = None                # cached compiled Bass program
LAST = None               # last BassKernelResults (for profiling)


def _build_mlp():
    """One SPMD program: per-core importance MLP over a K-shard.

    Features arrive pre-cast to bf16 (halves DMA traffic; empirically
    preserves the exact n_r rounding decisions, which have >=4e-3
    margin while bf16 share error is <8e-4). Matmuls run in bf16 on
    the PE with f32 PSUM accumulation; z is written out in f32.
    """
    from concourse import bacc, mybir, tile

    f32 = mybir.dt.float32
    bf16 = mybir.dt.bfloat16
    nc = bacc.Bacc(None, target_bir_lowering=False, debug=False)

    feat = nc.declare_dram_parameter("features", [B, C, KC], bf16, False)
    w1 = nc.declare_dram_parameter("W1", [C, H], bf16, False)
    b1 = nc.declare_dram_parameter("b1", [H, 1], f32, False)
    w2 = nc.declare_dram_parameter("W2", [H, 1], bf16, False)
    out = nc.declare_dram_parameter("z", [B, KC], f32, True)

    with tile.TileContext(nc) as tc:
        with (
            tc.tile_pool(name="wpool", bufs=1) as wpool,
            tc.tile_pool(name="fpool", bufs=1) as fpool,
            tc.tile_pool(name="hpool", bufs=4) as hpool,
            tc.tile_pool(name="ph", bufs=4, space="PSUM") as phpool,
            tc.tile_pool(name="pi", bufs=4, space="PSUM") as pipool,
        ):
            w1t = []
            for c in range(CH):
                t = wpool.tile([128, H], bf16, tag=f"w1_{c}")
                nc.sync.dma_start(out=t[:], in_=w1[c * 128:(c + 1) * 128, :])
                w1t.append(t)
            w2t = wpool.tile([H, 1], bf16, tag="w2")
            nc.sync.dma_start(out=w2t[:], in_=w2[:, :])
            b1t = wpool.tile([H, 1], f32, tag="b1")
            nc.sync.dma_start(out=b1t[:], in_=b1[:, :])

            # Stage the whole 4 MB bf16 feature shard in SBUF: one DMA
            # per (b, c-chunk, half) -> 8 KB contiguous rows.
            ftiles = {}
            for b in range(B):
                for c in range(CH):
                    t = fpool.tile([128, KC], bf16, tag=f"f_{b}_{c}")
                    for q in range(2):
                        sl = slice(q * (KC // 2), (q + 1) * (KC // 2))
                        nc.sync.dma_start(
                            out=t[:, sl],
                            in_=feat[b, c * 128:(c + 1) * 128, sl],
                        )
                    ftiles[b, c] = t

            relu = mybir.ActivationFunctionType.Relu
            for b in range(B):
                for i in range(KC // PT):
                    sl = slice(i * PT, (i + 1) * PT)
                    ph = phpool.tile([H, PT], f32)
                    for c in range(CH):
                        nc.tensor.matmul(
                            ph[:],
                            w1t[c][:],
                            ftiles[b, c][:, sl],
                            start=(c == 0),
                            stop=(c == CH - 1),
                        )
                    hs = hpool.tile([H, PT], bf16)
                    nc.scalar.activation(hs[:], ph[:], relu, bias=b1t[:])
                    pi = pipool.tile([1, PT], f32)
                    nc.tensor.matmul(pi[:], w2t[:], hs[:], start=True, stop=True)
                    oi = hpool.tile([1, PT], f32, tag="oi")
                    nc.vector.tensor_copy(oi[:], pi[:])
                    nc.sync.dma_start(out=out[b:b + 1, sl], in_=oi[:])

    nc.compile()
    return nc


def _get_nc():
    global _NC
    if _NC is None:
        _NC = _build_mlp()
    return _NC


def _ensure_profile_hook():
    """Shim antenv.axon_hooks (absent in this image) so the trace=True
    path of run_bass_kernel_spmd can capture NTFF profiles, and stub the
    S3 artifact upload. Only used when BASS_PROFILE=1."""
    import types

    try:
        import antenv.axon_hooks  # noqa: F401
    except ImportError:
        try:
            import antenv
            from trn_agent_boot.trn_boot import _ntff_profile_via_ctypes

            hook = _ntff_profile_via_ctypes("/opt/axon/libaxon_pjrt.so")
            mod = types.ModuleType("antenv.axon_hooks")
            mod.get_axon_ntff_profile_hook = lambda: hook
            mod.set_axon_ntff_profile_hook = lambda h: None
            sys.modules["antenv.axon_hooks"] = mod
            antenv.axon_hooks = mod
        except Exception:
            return False
    import concourse.bass_utils as bu

    bu.upload_artifacts = lambda d: "file://" + d
    return True


def _device_importance(features, W1, b1, W2, b2):
    from concourse.bass_utils import run_bass_kernel_spmd

    global LAST
    nc = _get_nc()
    profile = bool(int(os.environ.get("BASS_PROFILE", "0")))
    if profile:
        profile = _ensure_profile_hook()
    import ml_dtypes

    bf = ml_dtypes.bfloat16
    w1 = np.ascontiguousarray(W1).astype(bf)
    b1c = np.ascontiguousarray(b1, np.float32).reshape(H, 1)
    w2 = np.ascontiguousarray(W2).astype(bf).reshape(H, 1)
    fbf = np.asarray(features, np.float32).astype(bf)
    in_maps = []
    for core in range(NCORES):
        sl = slice(core * KC, (core + 1) * KC)
        in_maps.append({
            "features": np.ascontiguousarray(fbf[:, :, sl]),
            "W1": w1, "b1": b1c, "W2": w2,
        })
    res = run_bass_kernel_spmd(
        nc, in_maps, core_ids=list(range(NCORES)), trace=profile,
    )
    LAST = res
    z = np.concatenate([res.results[c]["z"] for c in range(NCORES)], axis=1)
    z = z + np.float32(np.asarray(b2).reshape(()))
    # softplus on host with the exact jax.nn.softplus formula in f32
    # (the ACT engine's Softplus LUT is not precise enough for the
    # data-dependent n_r rounding margins).
    return np.maximum(z, 0) + np.log1p(np.exp(-np.abs(z)))


def _fps_region(pts, n_steps):
    """Farthest-point sampling over one compacted region, mirroring the
    reference: start at subset index 0, squared L2, first-max argmax."""
    n = pts.shape[0]
    picks = np.empty(n_steps, np.int64)
    mind = np.full(n, np.float32(1e10), np.float32)
    p = 0
    x, y, z = pts[:, 0], pts[:, 1], pts[:, 2]
    for s in range(n_steps):
        picks[s] = p
        dx = x - x[p]
        dy = y - y[p]
        dz = z - z[p]
        d = (dx * dx + dy * dy) + dz * dz
        np.minimum(mind, d, out=mind)
        p = int(np.argmax(mind))
    return picks


def kernel(xyz, features, W1, b1, W2, b2, Wa, ba):
    xyz = np.asarray(xyz, np.float32)
    features = np.asarray(features, np.float32)
    Wa = np.asarray(Wa, np.float32)
    ba = np.asarray(ba, np.float32)

    imp = _device_importance(features, W1, b1, W2, b2)        # (B, K)

    # ---- spatial bucketing (exact reference semantics, f32 ops) ----
    mn = xyz.min(axis=1, keepdims=True)
    mx = xyz.max(axis=1, keepdims=True)
    xn = (xyz - mn) / (mx - mn + np.float32(1e-6))
    ridx = np.clip(xn * np.float32(RPD), 0, RPD - 1).astype(np.int32)
    rid = ridx[..., 0] * RPD * RPD + ridx[..., 1] * RPD + ridx[..., 2]
    valid = np.abs(xyz).sum(-1) > 0                           # (B, K)

    onehot = (rid[..., None] == np.arange(R3)) & valid[..., None]
    counts = onehot.sum(axis=1).astype(np.int32)              # (B, R3)
    reg_imp = np.einsum(
        "bk,bkr->br",
        (imp * valid).astype(np.float32),
        onehot.astype(np.float32),
    )
    share = reg_imp / (reg_imp.sum(-1, keepdims=True) + np.float32(1e-8))
    n_r = np.round(share * np.float32(T_TOK)).astype(np.int32)
    c_r = np.where(n_r == 0, 0, np.minimum(n_r, counts))      # (B, R3)

    # ---- per-region selection: FPS picks or ascending slab ----
    out_idx = np.zeros((B, T_TOK), np.int32)
    filled = np.zeros((B, T_TOK), bool)
    for b in range(B):
        start = 0
        for r in range(R3):
            c = int(c_r[b, r])
            if c == 0:
                continue
            members = np.nonzero(onehot[b, :, r])[0]          # ascending
            if counts[b, r] <= n_r[b, r]:
                sel = members[:c]
            else:
                pts = xyz[b][members]
                sel = members[_fps_region(pts, c)]
            take = min(c, T_TOK - start)
            if take > 0:
                out_idx[b, start:start + take] = sel[:take]
                filled[b, start:start + take] = True
            start += c
            if start >= T_TOK:
                break

    # ---- gather + output heads ----
    xyz_tok = np.where(
        filled[..., None], np.take_along_axis(xyz, out_idx[..., None], axis=1), 0.0
    ).astype(np.float32)

    gath = np.stack([features[b][:, out_idx[b]].T for b in range(B)])  # (B,T,C)
    tok = gath @ Wa + ba                                               # (B,T,D)
    feat_tok = np.where(filled[..., None], tok, 0.0).transpose(0, 2, 1)
    return xyz_tok.astype(np.float32), feat_tok.astype(np.float32)
